# revision 27
# baseline (speedup 1.0000x reference)
"""Trainium2 Bass kernel for NeuralGraphHidden (GNN message passing).

Full-input contract: kernel(**inputs) takes the complete unsharded arrays,
shards batch dim 0 across 8 NeuronCores (data parallel), runs one SPMD Bass
program, and reassembles the full output.

Math (per molecule, A=128 atoms, D=5 degree slots):
  deg[a]   = #(edges[a,:] != -1)
  out[a]   = relu(feat[a] @ W[deg[a]] + b[deg[a]])  if deg[a] < 5 else 0
  feat[a]  = [atoms[a] + sum_d atoms[edges[a,d]],  sum_d bonds[a,d]]

Degree-sparsity formulation: deg[a] == D (all 5 neighbour slots used) makes
every mask (deg == d), d < 5, zero -> the output row is structurally zero,
independent of any float input. For uniform random edges that is ~96% of
atoms. The kernel therefore:

  host (index metadata only, no float math):
    - per molecule, list the "selected" atoms (deg < 5; <= S=16 slots) and
      the "exception" atoms (deg < 4; <= E=4 slots)
    - build per-molecule gather/sum matrices ET[src, slot] (counts of each
      src atom in a slot's neighbour list, + self), slot-compacted
    - gather selected atoms' bond rows (pure fancy-indexing layout)
  device (all float compute):
    - featT[:, slot] = atoms_m.T @ ET_m          (TensorE, bf16)
    - bond sums for selected slots               (VectorE reduce)
    - Z4 = relu(featT.T @ Waug[4]) for all main slots   (TensorE)
    - full 5-degree Z + degree-select + relu for the exception tile
  host: scatter device rows into a zero output (zeros are determined by
    deg metadata alone; every value that depends on float data comes from
    the device).

A dense all-degrees fallback program (the previous kernel) is kept for the
(never observed) case that a molecule exceeds the slot capacities.
"""

import sys

sys.path.insert(0, "/opt/trn_rl_repo")

import numpy as np

B, A, D = 256, 128, 5
FA, FB, C = 256, 64, 256
F = FA + FB        # 320
FAUG = F + 1       # 321 (bias row)
NCORES = 8
BL = B // NCORES   # 32 molecules per core

S = 16             # main slots per molecule (deg<5 atoms), capacity
E = 4              # exception slots per molecule (deg<4 atoms), capacity
SLOT = S + E       # 20 psum columns per molecule
G = 8              # molecules per slot tile (8*16 = 128)
NT = BL // G       # 4 main slot tiles per core
MAIN = NT * G * S  # 512 main slots per core
EXC = BL * E       # 128 exception slots per core

_CACHE = {}


# ---------------------------------------------------------------------------
# sparse program
# ---------------------------------------------------------------------------

def _build_program_sparse(L, PACK):
    """L = #distinct exception degrees; PACK = molecules per 128-row atom bin."""
    from contextlib import ExitStack

    import concourse.bass as bass
    import concourse.tile as tile
    from concourse import bacc, mybir

    f32 = mybir.dt.float32
    AF = mybir.ActivationFunctionType
    OP = mybir.AluOpType
    bf16 = mybir.dt.bfloat16

    nc = bacc.Bacc("TRN2", target_bir_lowering=False, debug=False,
                   num_devices=NCORES)

    # gathered atoms: PACK molecules share a 128-partition bin (their
    # unique referenced atoms fit in 128 rows), partition-major layout
    NBINS = BL // PACK
    NBT = G // PACK            # bins per slot tile
    WC = (1 + L) * C           # weight columns: [d=4 | exc degrees]

    atoms_d = nc.dram_tensor("atoms", [A, NBINS * FA], bf16,
                             kind="ExternalInput")
    # per-tile gather/sum matrices (split so each tile's chain starts early)
    et_d = nc.dram_tensor("et", [NT, A, G * SLOT], bf16,
                          kind="ExternalInput")
    # per-tile bond rows, degree-major: [tile][64, d*128+slot]
    bondm_d = nc.dram_tensor("bondm", [NT, FB, D * A], bf16,
                             kind="ExternalInput")
    bonde_d = nc.dram_tensor("bonde", [FB, D * EXC], bf16,
                             kind="ExternalInput")
    # Waug = [W; b]: degree-4 block and exception-degrees block, contiguous
    wd4_d = nc.dram_tensor("wd4", [FAUG, C], bf16, kind="ExternalInput")
    wexc_d = nc.dram_tensor("wexc", [FAUG, max(L, 1) * C], bf16,
                            kind="ExternalInput")
    md_d = nc.dram_tensor("md", [EXC, max(L, 1) * EXC], bf16,
                          kind="ExternalInput")
    ones_d = nc.dram_tensor("ones", [1, MAIN], bf16, kind="ExternalInput")
    # bf16 outputs (host upcasts): halves the output DMA in the tail
    out_d = nc.dram_tensor("out", [NT + 1, A, C], bf16, kind="ExternalOutput")

    atoms_ap = atoms_d.ap()
    out_ap = out_d.ap()

    with tile.TileContext(nc) as tc, ExitStack() as ctx:
        consts = ctx.enter_context(tc.tile_pool(name="consts", bufs=1))
        pin = ctx.enter_context(tc.tile_pool(name="pin", bufs=1))
        pfeat = ctx.enter_context(tc.tile_pool(name="pfeat", bufs=1))
        pmisc = ctx.enter_context(tc.tile_pool(name="pmisc", bufs=1))
        pbs = ctx.enter_context(tc.tile_pool(name="pbs", bufs=4))
        pout = ctx.enter_context(tc.tile_pool(name="pout", bufs=3))
        ps_f = ctx.enter_context(
            tc.tile_pool(name="ps_f", bufs=4, space="PSUM"))
        ps_z = ctx.enter_context(
            tc.tile_pool(name="ps_z", bufs=2, space="PSUM"))
        ps_g = ctx.enter_context(
            tc.tile_pool(name="ps_g", bufs=1, space="PSUM"))

        # ---- input DMAs --------------------------------------------------
        # Only Sync/Scalar (HWDGE) and GpSimd (SWDGE) can issue DMAs; each
        # queue is FIFO at ~80-110 GB/s effective, so transfers are
        # need-ordered per queue and the per-tile chains interleave.
        WE = max(L, 1) * C
        et_t = [consts.tile([A, G * SLOT], bf16, tag=f"et{t}",
                            name=f"et{t}") for t in range(NT)]
        wd4 = [consts.tile([128, C], bf16, tag="wd40", name="wd40"),
               consts.tile([128, C], bf16, tag="wd41", name="wd41"),
               consts.tile([FAUG - 256, C], bf16, tag="wd42", name="wd42")]
        wexc = [consts.tile([128, WE], bf16, tag="wexc0", name="wexc0"),
                consts.tile([128, WE], bf16, tag="wexc1", name="wexc1"),
                consts.tile([FAUG - 256, WE], bf16, tag="wexc2",
                            name="wexc2")]
        atiles = [pin.tile([A, NBT * FA], bf16, tag=f"at{t}",
                           name=f"at{t}") for t in range(NT)]
        bondm_t = [pbs.tile([FB, D * A], bf16, tag=f"bm{t}",
                            name=f"bm{t}") for t in range(NT)]
        ones = consts.tile([1, MAIN], bf16)
        mexp = consts.tile([EXC, max(L, 1) * EXC], bf16)
        bonde = consts.tile([FB, D * EXC], bf16)

        def A_dma(eng, t):
            eng.dma_start(out=atiles[t][:],
                          in_=atoms_ap[:, t * NBT * FA:(t + 1) * NBT * FA])

        # sync queue
        nc.sync.dma_start(out=et_t[0][:], in_=et_d.ap()[0])
        A_dma(nc.sync, 0)
        nc.sync.dma_start(out=wd4[0][:], in_=wd4_d.ap()[0:128, :])
        A_dma(nc.sync, 2)
        nc.sync.dma_start(out=et_t[3][:], in_=et_d.ap()[3])
        nc.sync.dma_start(out=wexc[2][:], in_=wexc_d.ap()[256:FAUG, :])
        nc.sync.dma_start(out=wexc[1][:], in_=wexc_d.ap()[128:256, :])
        # scalar queue
        nc.scalar.dma_start(out=et_t[1][:], in_=et_d.ap()[1])
        A_dma(nc.scalar, 1)
        nc.scalar.dma_start(out=wd4[1][:], in_=wd4_d.ap()[128:256, :])
        A_dma(nc.scalar, 3)
        nc.scalar.dma_start(out=wexc[0][:], in_=wexc_d.ap()[0:128, :])
        # gpsimd queue
        nc.gpsimd.dma_start(out=ones[:], in_=ones_d.ap()[:])
        nc.gpsimd.dma_start(out=mexp[:], in_=md_d.ap()[:])
        nc.gpsimd.dma_start(out=bondm_t[0][:], in_=bondm_d.ap()[0])
        nc.gpsimd.dma_start(out=wd4[2][:], in_=wd4_d.ap()[256:FAUG, :])
        nc.gpsimd.dma_start(out=bondm_t[1][:], in_=bondm_d.ap()[1])
        nc.gpsimd.dma_start(out=et_t[2][:], in_=et_d.ap()[2])
        nc.gpsimd.dma_start(out=bondm_t[2][:], in_=bondm_d.ap()[2])
        nc.gpsimd.dma_start(out=bondm_t[3][:], in_=bondm_d.ap()[3])
        nc.gpsimd.dma_start(out=bonde[:], in_=bonde_d.ap()[:])

        # PE HAM warm-up: idle matmuls on a memset tile while inputs stream
        # in, so the real matmuls run at 2.4 GHz instead of 1.2
        warmsrc = consts.tile([1, 512], bf16)
        nc.vector.memset(warmsrc[:], 1.0)
        warm = ps_g.tile([A, 512], f32, tag="warm", bufs=1)
        for _ in range(6):
            nc.tensor.matmul(warm[:], warmsrc[:, 0:128], warmsrc[:])

        # bond-sum chunks (64 bond rows + bias-ones row), bf16 stationary
        chunk2m = pfeat.tile([FAUG - 256, MAIN], bf16)
        nc.vector.tensor_copy(chunk2m[FB:FB + 1, :], ones[:])
        chunk2e = pfeat.tile([FAUG - 256, EXC], bf16)
        nc.vector.tensor_copy(chunk2e[FB:FB + 1, :], ones[:, 0:EXC])

        def bond_tree(dst, raw, n):
            # dst[0:FB, :] = sum_d raw[:, d*n:(d+1)*n] (degree-major layout)
            t01 = pbs.tile([FB, A], bf16, tag="bt01")
            t23 = pbs.tile([FB, A], bf16, tag="bt23")
            with nc.allow_low_precision(reason="bf16 bond sums, rel ~4e-3"):
                nc.vector.tensor_add(t01[:, 0:n], raw[:, 0:n], raw[:, n:2 * n])
                nc.vector.tensor_add(t23[:, 0:n], raw[:, 2 * n:3 * n],
                                     raw[:, 3 * n:4 * n])
                nc.vector.tensor_add(t01[:, 0:n], t01[:, 0:n], t23[:, 0:n])
                nc.vector.tensor_add(dst, t01[:, 0:n], raw[:, 4 * n:5 * n])

        # compacted transposed features: [f-chunk, slot]
        featm0 = pfeat.tile([128, MAIN], bf16)
        featm1 = pfeat.tile([128, MAIN], bf16)
        fe0m = [pfeat.tile([128, EXC], bf16, name=f"fe0m{i}")
                for i in range(L)]
        fe1m = [pfeat.tile([128, EXC], bf16, name=f"fe1m{i}")
                for i in range(L)]
        c2em = [pfeat.tile([FAUG - 256, EXC], bf16, name=f"c2em{i}")
                for i in range(L)]

        # ---- neighbour+self sums, one slot tile per 8 molecules ----------
        for t in range(NT):
            bond_tree(chunk2m[0:FB, t * A:(t + 1) * A], bondm_t[t][:], A)

            pfb = ps_f.tile([A, 2 * G * SLOT], f32, tag="pf")
            pf0 = pfb[:, 0:G * SLOT]
            pf1 = pfb[:, G * SLOT:2 * G * SLOT]
            W_B = PACK * SLOT             # psum columns per bin
            for bb in range(NBT):         # molecule bins within the tile
                bn = t * NBT + bb
                atile = atiles[t]
                ecols = et_t[t][:, bb * W_B:(bb + 1) * W_B]
                nc.tensor.matmul(pf0[:, bb * W_B:(bb + 1) * W_B],
                                 atile[:, bb * FA:bb * FA + 128], ecols)
                nc.tensor.matmul(pf1[:, bb * W_B:(bb + 1) * W_B],
                                 atile[:, bb * FA + 128:(bb + 1) * FA], ecols)

            with nc.allow_low_precision(reason="bf16 features, rel ~4e-3"):
                # main slots (16 per molecule) -> featm tiles
                nc.scalar.copy(
                    featm0[:, t * A:(t + 1) * A].rearrange(
                        "p (j k) -> p j k", j=G),
                    pf0.rearrange("p (j k) -> p j k", j=G)[:, :, 0:S])
                nc.scalar.copy(
                    featm1[:, t * A:(t + 1) * A].rearrange(
                        "p (j k) -> p j k", j=G),
                    pf1.rearrange("p (j k) -> p j k", j=G)[:, :, 0:S])
                # exception slots (4 per molecule) -> fe tiles, pre-masked
                # per present degree (mexp is a 0/1 broadcast mask, so the
                # per-degree select happens here instead of extra matmuls)
                e0, e1 = t * G * E, (t + 1) * G * E
                for i in range(L):
                    mcols = mexp[:, i * EXC + e0:i * EXC + e1]
                    nc.vector.tensor_mul(
                        fe0m[i][:, e0:e1].rearrange("p (j k) -> p j k", j=G),
                        pf0.rearrange("p (j k) -> p j k", j=G)[:, :, S:SLOT],
                        mcols.rearrange("p (j k) -> p j k", j=G))
                    nc.vector.tensor_mul(
                        fe1m[i][:, e0:e1].rearrange("p (j k) -> p j k", j=G),
                        pf1.rearrange("p (j k) -> p j k", j=G)[:, :, S:SLOT],
                        mcols.rearrange("p (j k) -> p j k", j=G))

            # ---- degree-4 dense for this tile's 128 main slots -----------
            pz = ps_z.tile([A, C], f32, tag="pz")
            nc.tensor.matmul(pz[:], featm0[:, t * A:(t + 1) * A],
                             wd4[0][:], start=True, stop=False)
            nc.tensor.matmul(pz[:], featm1[:, t * A:(t + 1) * A],
                             wd4[1][:], start=False, stop=False)
            nc.tensor.matmul(pz[:], chunk2m[:, t * A:(t + 1) * A],
                             wd4[2][:], start=False, stop=True)
            outt = pout.tile([A, C], bf16)
            with nc.allow_low_precision(reason="bf16 output, rel ~4e-3"):
                nc.scalar.activation(outt[:], pz[:], AF.Relu)
            eng = nc.gpsimd if t % 2 == 0 else nc.scalar
            eng.dma_start(out=out_ap[t], in_=outt[:])

        # ---- exception tile: masked per-degree dense, one accumulator ----
        if L > 0:
            bond_tree(chunk2e[0:FB, :], bonde[:], EXC)
            with nc.allow_low_precision(reason="exact 0/1 masking"):
                for i in range(L):
                    nc.vector.tensor_mul(c2em[i][:], chunk2e[:],
                                         mexp[0:FAUG - 256,
                                              i * EXC:(i + 1) * EXC])

            pz = ps_z.tile([EXC, C], f32, tag="pz")
            for i in range(L):
                nc.tensor.matmul(pz[:], fe0m[i][:], wexc[0][:, i * C:(i + 1) * C],
                                 start=(i == 0), stop=False)
                nc.tensor.matmul(pz[:], fe1m[i][:], wexc[1][:, i * C:(i + 1) * C],
                                 start=False, stop=False)
                nc.tensor.matmul(pz[:], c2em[i][:], wexc[2][:, i * C:(i + 1) * C],
                                 start=False, stop=(i == L - 1))
            outt = pout.tile([EXC, C], bf16)
            with nc.allow_low_precision(reason="bf16 output, rel ~4e-3"):
                nc.scalar.activation(outt[:], pz[:], AF.Relu)
            nc.sync.dma_start(out=out_ap[NT], in_=outt[:])

    nc.compile()
    return nc


def _sparse_metadata(edges, PACK):
    """Host-side index metadata: slot assignment + gather/sum matrices.

    PACK consecutive molecules share one 128-row gathered-atom bin.
    Returns None if any molecule exceeds the S/E slot capacities or a bin
    overflows 128 unique referenced atoms (caller tries smaller PACK, then
    the dense fallback).
    """
    import ml_dtypes

    deg = (edges != -1).sum(axis=2)                      # (B, A)
    sel_mask = deg < D
    exc_mask = deg < D - 1
    if sel_mask.sum(axis=1).max() > S or exc_mask.sum(axis=1).max() > E:
        return None

    bf = ml_dtypes.bfloat16
    NBINS = BL // PACK
    et = np.zeros((NCORES, A, BL * SLOT), dtype=np.float32)
    # gather: atoms tile row (bin, local) -> flat (mol*A + atom)
    gidx = np.zeros((NCORES, NBINS, A), dtype=np.int64)
    # (core, slot) -> (molecule-in-core, atom) for output scatter
    main_rows = [[] for _ in range(NCORES)]
    exc_rows = [[] for _ in range(NCORES)]
    bidx_m = np.zeros((NCORES, MAIN), dtype=np.int64)    # flat (mol*A+atom)
    bidx_e = np.zeros((NCORES, EXC), dtype=np.int64)
    bval_m = np.zeros((NCORES, MAIN), dtype=bool)
    bval_e = np.zeros((NCORES, EXC), dtype=bool)

    for bm in range(B):
        c, ml = divmod(bm, BL)
        sel = np.nonzero(sel_mask[bm])[0]
        # bin-local gather table: selected atoms + their neighbours
        if ml % PACK == 0:
            _binoff = 0
        uniq = []
        loc = {}
        for a in sel:
            for src in [a] + [e for e in edges[bm, a] if e >= 0]:
                if src not in loc:
                    loc[src] = len(uniq)
                    uniq.append(src)
        half = _binoff
        _binoff += len(uniq)
        if _binoff > A:
            return None
        bn = ml // PACK
        for i, src in enumerate(uniq):
            gidx[c, bn, half + i] = ml * A + src
        for k, a in enumerate(sel):
            col = ml * SLOT + k
            et[c, half + loc[a], col] += 1.0             # self
            for e in edges[bm, a]:
                if e >= 0:
                    et[c, half + loc[e], col] += 1.0
            t, j = divmod(ml, G)
            slot = t * A + j * S + k
            main_rows[c].append((slot, ml, a))
            bidx_m[c, slot] = ml * A + a
            bval_m[c, slot] = True
            if deg[bm, a] < D - 1:
                # per-molecule exception slot index
                ke = sum(1 for s in exc_rows[c] if s[1] == ml)
                eslot = ml * E + ke
                col2 = ml * SLOT + S + ke
                et[c, half + loc[a], col2] += 1.0
                for e in edges[bm, a]:
                    if e >= 0:
                        et[c, half + loc[e], col2] += 1.0
                exc_rows[c].append((eslot, ml, a))
                bidx_e[c, eslot] = ml * A + a
                bval_e[c, eslot] = True

    # distinct exception degrees across all cores -> weight/select layout
    dlist = sorted({int(d) for c in range(NCORES)
                    for _, ml, a in exc_rows[c]
                    for d in [deg[c * BL + ml, a]]})
    L = len(dlist)
    md = np.zeros((NCORES, EXC, max(L, 1) * EXC), dtype=np.float32)
    for c in range(NCORES):
        for eslot, ml, a in exc_rows[c]:
            i = dlist.index(int(deg[c * BL + ml, a]))
            md[c, :, i * EXC + eslot] = 1.0    # broadcast mask column

    return {
        "et": et.astype(bf),
        "dlist": dlist,
        "md": md.astype(bf),
        "gidx": gidx,
        "main_rows": main_rows,
        "exc_rows": exc_rows,
        "bidx_m": bidx_m,
        "bval_m": bval_m,
        "bidx_e": bidx_e,
        "bval_e": bval_e,
    }


def _make_in_maps_sparse(atoms, bonds, W, b, meta):
    import ml_dtypes

    bf = ml_dtypes.bfloat16
    atoms_flat = atoms.reshape(NCORES, BL * A, FA)
    # gathered per molecule pair, partition-major: [A, NPAIR*FA]
    atoms8 = np.ascontiguousarray(
        atoms_flat[np.arange(NCORES)[:, None, None],
                   meta["gidx"]].transpose(0, 2, 1, 3).reshape(
                       NCORES, A, -1)).astype(bf)
    waug = np.concatenate([W, b[:, None, :]], axis=1)     # (5, 321, 256)
    wd4 = np.ascontiguousarray(waug[4]).astype(bf)        # (321, 256)
    L = len(meta["dlist"])
    if L:
        wexc = np.ascontiguousarray(
            waug[meta["dlist"]].transpose(1, 0, 2).reshape(
                FAUG, L * C)).astype(bf)
    else:
        wexc = np.zeros((FAUG, C), dtype=bf)
    ones = np.ones((1, MAIN), dtype=bf)

    # bond rows for selected slots, transposed to [64, slots*D]
    bonds_flat = bonds.reshape(NCORES, BL * A, D, FB)
    in_maps = []
    for c in range(NCORES):
        bm = bonds_flat[c][meta["bidx_m"][c]]             # (MAIN, D, FB)
        bm = bm * meta["bval_m"][c][:, None, None]
        be = bonds_flat[c][meta["bidx_e"][c]]
        be = be * meta["bval_e"][c][:, None, None]
        # per-tile, degree-major: [NT][FB, d*128 + slot]
        NTt = MAIN // A
        bondm = np.ascontiguousarray(
            bm.reshape(NTt, A, D, FB).transpose(0, 3, 2, 1).reshape(
                NTt, FB, D * A)).astype(bf)
        bonde = np.ascontiguousarray(
            be.transpose(2, 1, 0).reshape(FB, D * EXC)).astype(bf)
        et_tiled = np.ascontiguousarray(
            meta["et"][c].reshape(A, NT, G * SLOT).transpose(1, 0, 2))
        in_maps.append({
            "atoms": atoms8[c],
            "et": et_tiled,
            "bondm": bondm,
            "bonde": bonde,
            "wd4": wd4,
            "wexc": wexc,
            "md": meta["md"][c],
            "ones": ones,
        })
    return in_maps


# ---------------------------------------------------------------------------
# dense fallback program (previous kernel, unchanged)
# ---------------------------------------------------------------------------

def _build_program_dense():
    from contextlib import ExitStack

    import concourse.bass as bass
    import concourse.tile as tile
    from concourse import bacc, mybir

    f32 = mybir.dt.float32
    AF = mybir.ActivationFunctionType
    OP = mybir.AluOpType
    f32r = mybir.dt.float32r
    bf16 = mybir.dt.bfloat16

    nc = bacc.Bacc("TRN2", target_bir_lowering=False, debug=False,
                   num_devices=NCORES)

    atoms_d = nc.dram_tensor("atoms", [BL, A, FA], f32r, kind="ExternalInput")
    bonds_d = nc.dram_tensor("bonds", [BL, A, D * FB], f32,
                             kind="ExternalInput")
    edges_d = nc.dram_tensor("edges", [BL, A, A * D], bf16,
                             kind="ExternalInput")
    waug_d = nc.dram_tensor("waug", [D, FAUG, C], f32r, kind="ExternalInput")
    ident_d = nc.dram_tensor("ident", [A, A], f32, kind="ExternalInput")
    identr_d = nc.dram_tensor("identr", [A, A], f32r, kind="ExternalInput")
    identb_d = nc.dram_tensor("identb", [A, A], bf16, kind="ExternalInput")
    iota_d = nc.dram_tensor("iota", [A, 1], f32, kind="ExternalInput")
    edeg_d = nc.dram_tensor("edeg", [BL, A, D], f32, kind="ExternalInput")
    onesr_d = nc.dram_tensor("onesr", [1, A], f32, kind="ExternalInput")
    out_d = nc.dram_tensor("out", [BL, A, C], f32, kind="ExternalOutput")

    atoms_ap = atoms_d.ap()
    bonds_ap = bonds_d.ap()
    edges_ap = edges_d.ap()
    out_ap = out_d.ap()

    with tile.TileContext(nc) as tc, ExitStack() as ctx:
        consts = ctx.enter_context(tc.tile_pool(name="consts", bufs=1))
        pin = ctx.enter_context(tc.tile_pool(name="pin", bufs=3))
        pbc = ctx.enter_context(tc.tile_pool(name="pbc", bufs=2))
        pet = ctx.enter_context(tc.tile_pool(name="pet", bufs=2))
        pfeat = ctx.enter_context(tc.tile_pool(name="pfeat", bufs=2))
        pmd = ctx.enter_context(tc.tile_pool(name="pmd", bufs=2))
        pz = ctx.enter_context(tc.tile_pool(name="pz", bufs=2))
        pout = ctx.enter_context(tc.tile_pool(name="pout", bufs=3))
        ps_f = ctx.enter_context(
            tc.tile_pool(name="ps_f", bufs=2, space="PSUM"))
        ps_c2 = ctx.enter_context(
            tc.tile_pool(name="ps_c2", bufs=1, space="PSUM"))
        ps_z = ctx.enter_context(
            tc.tile_pool(name="ps_z", bufs=1, space="PSUM"))
        ps_s = ctx.enter_context(
            tc.tile_pool(name="ps_s", bufs=1, space="PSUM"))

        G4 = 4
        ident = consts.tile([A, A], f32)
        nc.scalar.dma_start(out=ident[:], in_=ident_d.ap()[:])
        identr = consts.tile([A, A], f32r)
        nc.scalar.dma_start(out=identr[:], in_=identr_d.ap()[:])
        iota_col = consts.tile([A, 1], f32)
        nc.gpsimd.dma_start(out=iota_col[:], in_=iota_d.ap()[:])
        ones_row = consts.tile([1, A], f32)
        nc.scalar.dma_start(out=ones_row[:], in_=onesr_d.ap()[:])
        identb4 = consts.tile([A, G4 * A], bf16)
        for j in range(G4):
            nc.gpsimd.dma_start(out=identb4[:, j * A:(j + 1) * A],
                                in_=identb_d.ap()[:])

        w0 = consts.tile([128, D * C], f32r)
        w1 = consts.tile([128, D * C], f32r)
        w2 = consts.tile([FAUG - 256, D * C], f32r)
        for d in range(D):
            nc.scalar.dma_start(out=w0[:, d * C:(d + 1) * C],
                                in_=waug_d.ap()[d, 0:128, :])
            nc.scalar.dma_start(out=w1[:, d * C:(d + 1) * C],
                                in_=waug_d.ap()[d, 128:256, :])
            nc.scalar.dma_start(out=w2[:, d * C:(d + 1) * C],
                                in_=waug_d.ap()[d, 256:FAUG, :])

        for bg in range(BL // G4):
            mols = range(bg * G4, (bg + 1) * G4)
            atoms4 = pin.tile([A, G4 * FA], f32r)
            nc.sync.dma_start(
                out=atoms4.rearrange("p (g f) -> p g f", g=G4),
                in_=atoms_ap[bg * G4:(bg + 1) * G4].rearrange(
                    "g p f -> p g f"))
            bonds4 = pin.tile([A, G4 * D * FB], f32)
            nc.sync.dma_start(
                out=bonds4.rearrange("p (g f) -> p g f", g=G4),
                in_=bonds_ap[bg * G4:(bg + 1) * G4].rearrange(
                    "g p f -> p g f"))
            bc_e4 = pbc.tile([A, G4 * A * D], bf16)
            nc.gpsimd.dma_start(
                out=bc_e4.rearrange("p (g f) -> p g f", g=G4),
                in_=edges_ap[bg * G4:(bg + 1) * G4].rearrange(
                    "g p f -> p g f"))
            edeg4 = pfeat.tile([A, G4 * D], f32)
            nc.sync.dma_start(
                out=edeg4.rearrange("p (g f) -> p g f", g=G4),
                in_=edeg_d.ap()[bg * G4:(bg + 1) * G4].rearrange(
                    "g p f -> p g f"))
            ne4 = pfeat.tile([A, G4 * D], f32)
            nc.vector.tensor_scalar(ne4[:], edeg4[:], -1.0, None,
                                    OP.not_equal)
            degp1_4 = pfeat.tile([A, G4], f32)
            nc.vector.tensor_reduce(
                degp1_4[:], ne4.rearrange("p (g d) -> p g d", g=G4),
                axis=mybir.AxisListType.X, op=OP.add)
            nc.vector.tensor_scalar(degp1_4[:], degp1_4[:], 1.0, None,
                                    OP.add)

            cmp5 = pbc.tile([A, G4 * A * D], bf16)
            nc.vector.tensor_scalar(cmp5[:], bc_e4[:], iota_col[:], None,
                                    OP.is_equal)
            cg = cmp5.rearrange("p (g d a) -> p g d a", g=G4, d=D)
            t01 = pet.tile([A, G4 * A], bf16)
            nc.vector.tensor_add(t01[:], cg[:, :, 0, :], cg[:, :, 1, :])
            t23 = pet.tile([A, G4 * A], bf16)
            nc.vector.tensor_add(t23[:], cg[:, :, 2, :], cg[:, :, 3, :])
            t4i = pet.tile([A, G4 * A], bf16)
            nc.vector.tensor_add(t4i[:], cg[:, :, 4, :], identb4[:])
            t0123 = pet.tile([A, G4 * A], bf16)
            nc.vector.tensor_add(t0123[:], t01[:], t23[:])
            etp4 = pet.tile([A, G4 * A], f32r)
            with nc.allow_low_precision(reason="exact small-int counts"):
                nc.vector.tensor_add(etp4[:], t0123[:], t4i[:])

            out4 = pout.tile([A, G4 * C], f32)
            for j, bm in enumerate(mols):
                etp = etp4[:, j * A:(j + 1) * A]
                atoms_sb = atoms4[:, j * FA:(j + 1) * FA]
                bonds_sb = bonds4[:, j * D * FB:(j + 1) * D * FB]

                degp1 = degp1_4[:, j:j + 1]

                pf = ps_f.tile([A, FA], f32)
                nc.tensor.matmul(pf[:, 0:128], atoms_sb[:, 0:128], etp)
                nc.tensor.matmul(pf[:, 128:256], atoms_sb[:, 128:256], etp)

                featT01 = pfeat.tile([A, FA], f32r)
                nc.scalar.copy(featT01[:], pf[:, 0:FA])

                sumbond = pfeat.tile([A, FB], f32r)
                with nc.allow_low_precision(
                        reason="f32r rounding of bond sums"):
                    nc.vector.reduce_sum(
                        sumbond[:],
                        bonds_sb.rearrange("p (d f) -> p f d", d=D),
                        axis=mybir.AxisListType.X)
                pc2 = ps_c2.tile([FB, A], f32)
                nc.tensor.matmul(pc2[:], sumbond[:], identr[:])
                chunk2 = pfeat.tile([FAUG - 256, A], f32r)
                nc.scalar.copy(chunk2[0:FB, :], pc2[:])
                nc.vector.tensor_copy(chunk2[FB:FB + 1, :], ones_row[:])

                md = pmd.tile([A, D * A], f32r)
                for d in range(D):
                    nc.vector.tensor_scalar(md[:, d * A:(d + 1) * A],
                                            ident[:], degp1[:], float(d + 1),
                                            OP.mult, OP.is_equal)

                lhs = [featT01[:, 0:128], featT01[:, 128:256], chunk2[:]]
                rhs = [w0, w1, w2]
                groups = [(0, 512), (512, 1024), (1024, 1280)]
                zsb = pz.tile([A, D * C], f32r)
                for g0, g1 in groups:
                    pzg = ps_z.tile([A, 512], f32, tag="pzg", bufs=4)
                    nc.tensor.matmul(pzg[:, 0:g1 - g0], lhs[0],
                                     rhs[0][:, g0:g1], start=True, stop=False)
                    nc.tensor.matmul(pzg[:, 0:g1 - g0], lhs[1],
                                     rhs[1][:, g0:g1], start=False,
                                     stop=False)
                    nc.tensor.matmul(pzg[:, 0:g1 - g0], lhs[2],
                                     rhs[2][:, g0:g1], start=False, stop=True)
                    nc.scalar.copy(zsb[:, g0:g1], pzg[:, 0:g1 - g0])

                pst = ps_s.tile([A, C], f32)
                for d in range(D):
                    nc.tensor.matmul(pst[:], md[:, d * A:(d + 1) * A],
                                     zsb[:, d * C:(d + 1) * C],
                                     start=(d == 0), stop=(d == D - 1))
                nc.scalar.activation(out4[:, j * C:(j + 1) * C], pst[:],
                                     AF.Relu)
            nc.gpsimd.dma_start(
                out=out_ap[bg * G4:(bg + 1) * G4].rearrange("g p f -> p g f"),
                in_=out4.rearrange("p (g f) -> p g f", g=G4))

    nc.compile()
    return nc


def _make_in_maps_dense(atoms, bonds, edges, W, b):
    atoms = np.ascontiguousarray(np.asarray(atoms, dtype=np.float32))
    bonds = np.ascontiguousarray(np.asarray(bonds, dtype=np.float32))
    edges = np.asarray(edges)
    W = np.asarray(W, dtype=np.float32)
    b = np.asarray(b, dtype=np.float32)

    import ml_dtypes
    edges_f = np.ascontiguousarray(edges.transpose(0, 2, 1)).reshape(
        B, D * A).astype(ml_dtypes.bfloat16)
    edges_rep = np.ascontiguousarray(
        np.broadcast_to(edges_f[:, None, :], (B, A, D * A)))

    waug = np.ascontiguousarray(
        np.concatenate([W, b[:, None, :]], axis=1))           # (5, 321, 256)
    ident = np.eye(A, dtype=np.float32)
    iota = np.arange(A, dtype=np.float32).reshape(A, 1)
    onesr = np.ones((1, A), dtype=np.float32)

    edeg8 = edges.reshape(NCORES, BL, A, D).astype(np.float32)
    atoms8 = atoms.reshape(NCORES, BL, A, FA)
    bonds8 = bonds.reshape(NCORES, BL, A, D * FB)
    edges8 = edges_rep.reshape(NCORES, BL, A, A * D)

    return [
        {
            "atoms": atoms8[c],
            "bonds": bonds8[c],
            "edges": edges8[c],
            "waug": waug,
            "ident": ident,
            "identr": ident,
            "identb": ident.astype(ml_dtypes.bfloat16),
            "iota": iota,
            "edeg": edeg8[c],
            "onesr": onesr,
        }
        for c in range(NCORES)
    ]


# ---------------------------------------------------------------------------
# entry points
# ---------------------------------------------------------------------------

def run_sharded(atoms, bonds, edges, W, b, trace=False):
    """Run on the 8 NeuronCores; returns (output, BassKernelResults)."""
    from concourse.bass_utils import run_bass_kernel_spmd

    atoms = np.ascontiguousarray(np.asarray(atoms, dtype=np.float32))
    bonds = np.ascontiguousarray(np.asarray(bonds, dtype=np.float32))
    edges = np.asarray(edges)
    W = np.asarray(W, dtype=np.float32)
    b = np.asarray(b, dtype=np.float32)

    meta = _sparse_metadata(edges, 4)
    PACK = 4
    if meta is None:
        meta = _sparse_metadata(edges, 2)
        PACK = 2
    if meta is None:
        if "dense" not in _CACHE:
            _CACHE["dense"] = _build_program_dense()
        nc = _CACHE["dense"]
        in_maps = _make_in_maps_dense(atoms, bonds, edges, W, b)
        res = run_bass_kernel_spmd(nc, in_maps, list(range(NCORES)),
                                   trace=trace)
        out = np.concatenate(
            [res.results[c]["out"] for c in range(NCORES)],
            axis=0).reshape(B, A, C)
        return out, res

    L = len(meta["dlist"])
    key = ("sparse", L, PACK)
    if key not in _CACHE:
        _CACHE[key] = _build_program_sparse(L, PACK)
    nc = _CACHE[key]
    in_maps = _make_in_maps_sparse(atoms, bonds, W, b, meta)
    res = run_bass_kernel_spmd(nc, in_maps, list(range(NCORES)), trace=trace)

    out = np.zeros((B, A, C), dtype=np.float32)
    for c in range(NCORES):
        dev = np.asarray(res.results[c]["out"], dtype=np.float32)
        main = dev[:NT].reshape(NT * A, C)         # (NT+1, 128, 256)
        mr = meta["main_rows"][c]
        er = meta["exc_rows"][c]
        exc_atoms = {(ml, a) for _, ml, a in er}
        for slot, ml, a in mr:
            if (ml, a) not in exc_atoms:
                out[c * BL + ml, a] = main[slot]
        exc = dev[NT]
        for eslot, ml, a in er:
            out[c * BL + ml, a] = exc[eslot]
    return out, res


def kernel(atoms, bonds, edges, W, b):
    out, _ = run_sharded(atoms, bonds, edges, W, b)
    return out
